# revision 21
# baseline (speedup 1.0000x reference)
"""
Trainium2 Bass kernel for nn_MetaAttention.

Computation (per batch b):
    rowsum[h,i]     = sum_j m[b,h,i,j]
    aggregated[i,j] = sum_h rowsum[h,i] * m[b,h,i,j]
    out[b]          = softmax(aggregated.flatten()).reshape(N, N)

Sharding: pure data parallel over B=16 across 8 cores (2 batches/core).

Per-core strategy (memory regime, ~64 MB HBM traffic/core):
  - Row tiles of P=112 partitions; partition p holds CONTIGUOUS rows
    7p..7p+6 ("(p t) j"), so each (b,h) is ONE 2.4 MB DMA whose
    per-partition descriptor is a single 21952 B contiguous DRAM
    segment (near-peak HBM streaming). Row permutation is transparent:
    the math is row-independent and the store inverts the mapping.
  - The scale-and-accumulate over heads is split across engine paths
    (fp32 PE matmul is 2-pass + half-rate, so PE alone can't carry it):
      * PE path (tiles k<4 only — PSUM holds 4 tiles = 8 banks):
        lhsT=diag(rowsum) matmul accumulating into PSUM; two heads per
        tile are skipped (rotating pairs) to even per-step load.
      * DVE path: fused scalar_tensor_tensor agg = m*rs + agg in SBUF.
      * GPSIMD path: ACT computes m*rs, gpsimd adds into agg.
    A DVE add merges each PSUM partial into the SBUF agg.
  - rowsums: heads alternate between one multi-tile DVE tensor_reduce
    ([P,7,784] -> [P,7] in one op) and per-tile ACT activation+accum.
  - Global softmax: per-tile max (DVE), cross-partition max/sum via PE
    transpose + all-ones matmul broadcast, exp with fused sums on ACT,
    final scale on ACT, one DMA store per batch (SWDGE queue).
"""

import numpy as np

B, H, N = 16, 12, 784
NCORES = 8
BPC = B // NCORES          # batches per core
P = 112                    # partition tile (784 = 7 * 112)
NT = N // P                # 7 row tiles
NPE = 4                    # tiles 0..NPE-1 accumulate on the PE path (PSUM)
JSPLITS = [(0, 512), (512, 272)]  # matmul free-dim splits (PSUM bank aligned)

# For PE-capable tile k (<NPE), these two heads go to SBUF paths instead,
# rotating so each h-step has a near-constant engine mix.
SKIP_PAIRS = {0: (10, 11), 1: (8, 9), 2: (6, 7), 3: (4, 5)}
ROWSUM_DVE_H = {0, 2, 4, 6, 8, 10}   # one multi-tile DVE reduce per (b,h)
GPS_EVEN_K = {0, 2}                  # skipped-pair 2nd unit -> GPS for these k

LAST_RESULT = None  # BassKernelResults of the most recent kernel() call


def build_program():
    import concourse.bacc as bacc
    import concourse.tile as tile
    from concourse import mybir

    f32 = mybir.dt.float32
    nc = bacc.Bacc("TRN2")

    x = nc.dram_tensor("x", [BPC, H, N, N], f32, kind="ExternalInput")
    ident = nc.dram_tensor("ident", [P, P], f32, kind="ExternalInput")
    y = nc.dram_tensor("y", [BPC, N, N], f32, kind="ExternalOutput")

    # path of unit (h, k): 'pe' | 'init' | 'dve' | 'gps'
    def path(h, k):
        if k < NPE:
            lo, hi = SKIP_PAIRS[k]
            if h == lo:
                return "init"
            if h == hi:
                return "gps" if k in GPS_EVEN_K else "dve"
            return "pe"
        if h == 0:
            return "init"
        return "gps" if h % 2 == 1 else "dve"

    with tile.TileContext(nc) as tc:
        with (
            tc.tile_pool(name="mh", bufs=4) as mh_pool,
            tc.tile_pool(name="agg", bufs=2) as agg_pool,
            tc.tile_pool(name="acc", bufs=4, space="PSUM") as acc_pool,
            tc.tile_pool(name="diag", bufs=4) as diag_pool,
            tc.tile_pool(name="scratch", bufs=4) as scratch_pool,
            tc.tile_pool(name="small", bufs=8) as small_pool,
            tc.tile_pool(name="consts", bufs=1) as const_pool,
        ):
            ident_sb = const_pool.tile([P, P], f32)
            nc.sync.dma_start(out=ident_sb, in_=ident[:, :])
            ones_sb = const_pool.tile([P, P], f32)
            nc.vector.memset(ones_sb, 1.0)

            for b in range(BPC):
                agg = agg_pool.tile([P, NT, N], f32, tag="agg")
                maxs = small_pool.tile([P, NT], f32, tag="maxs")
                sums = small_pool.tile([P, NT], f32, tag="sums")
                accs = [
                    acc_pool.tile([P, 1024], f32, tag="acc", name=f"acc_{b}_{k}")
                    for k in range(NPE)
                ]

                pe_first = {k: min(h for h in range(H) if path(h, k) == "pe")
                            for k in range(NPE)}
                pe_last = {k: max(h for h in range(H) if path(h, k) == "pe")
                           for k in range(NPE)}

                for h in range(H):
                    mh = mh_pool.tile([P, NT, N], f32, tag="mh")
                    # partition p <- contiguous rows 7p..7p+6 of m[b,h]
                    src = x[b, h].rearrange("(p t) j -> p t j", p=P)
                    nc.sync.dma_start(out=mh, in_=src)

                    if h in ROWSUM_DVE_H:
                        rs7 = small_pool.tile([P, NT], f32, tag="rs7")
                        nc.vector.tensor_reduce(
                            out=rs7, in_=mh, axis=mybir.AxisListType.X,
                            op=mybir.AluOpType.add,
                        )
                        rs_of = lambda k: rs7[:, k : k + 1]
                    else:
                        rss = []
                        for k in range(NT):
                            rsk = small_pool.tile([P, 1], f32, tag="rs",
                                                  name=f"rs_{b}_{h}_{k}")
                            scr = scratch_pool.tile([P, N], f32, tag="scr",
                                                    name=f"scr_{b}_{h}_{k}")
                            nc.scalar.activation(
                                out=scr, in_=mh[:, k, :],
                                func=mybir.ActivationFunctionType.Copy,
                                bias=0.0, scale=1.0, accum_out=rsk,
                            )
                            rss.append(rsk)
                        rs_of = lambda k: rss[k]

                    for k in range(NT):
                        rs = rs_of(k)
                        p_ = path(h, k)
                        if p_ == "pe":
                            dg = diag_pool.tile([P, P], f32, tag="dg")
                            nc.vector.tensor_scalar_mul(
                                out=dg, in0=ident_sb, scalar1=rs
                            )
                            for j0, jn in JSPLITS:
                                nc.tensor.matmul(
                                    accs[k][:, j0 : j0 + jn],
                                    lhsT=dg,
                                    rhs=mh[:, k, j0 : j0 + jn],
                                    start=(h == pe_first[k]),
                                    stop=(h == pe_last[k]),
                                )
                        elif p_ == "init":
                            nc.vector.tensor_scalar_mul(
                                out=agg[:, k, :], in0=mh[:, k, :], scalar1=rs
                            )
                        elif p_ == "dve":
                            nc.vector.scalar_tensor_tensor(
                                out=agg[:, k, :],
                                in0=mh[:, k, :],
                                scalar=rs,
                                in1=agg[:, k, :],
                                op0=mybir.AluOpType.mult,
                                op1=mybir.AluOpType.add,
                            )
                        else:  # gps: scale on ACT, add on gpsimd
                            sc2 = scratch_pool.tile([P, N], f32, tag="sc2")
                            nc.scalar.activation(
                                out=sc2, in_=mh[:, k, :],
                                func=mybir.ActivationFunctionType.Copy,
                                bias=0.0, scale=rs,
                            )
                            nc.gpsimd.tensor_tensor(
                                out=agg[:, k, :],
                                in0=sc2,
                                in1=agg[:, k, :],
                                op=mybir.AluOpType.add,
                            )

                # merge PSUM partials into agg; per-tile max
                for k in range(NT):
                    if k < NPE:
                        nc.vector.tensor_add(
                            out=agg[:, k, :],
                            in0=agg[:, k, :],
                            in1=accs[k][:, 0:N],
                        )
                    nc.vector.tensor_reduce(
                        out=maxs[:, k : k + 1],
                        in_=agg[:, k, :],
                        axis=mybir.AxisListType.X,
                        op=mybir.AluOpType.max,
                    )

                # ---- softmax over the full [N, N] of this batch ----
                m1 = small_pool.tile([P, 1], f32, tag="m1")
                nc.vector.tensor_reduce(
                    out=m1, in_=maxs, axis=mybir.AxisListType.X,
                    op=mybir.AluOpType.max,
                )
                tps = acc_pool.tile([1, P], f32, tag="acc", name=f"tps_{b}")
                nc.tensor.transpose(tps, m1, ident_sb)
                gm = small_pool.tile([1, 1], f32, tag="gm")
                nc.vector.tensor_reduce(
                    out=gm, in_=tps, axis=mybir.AxisListType.X,
                    op=mybir.AluOpType.max,
                )
                bps = acc_pool.tile([P, 1], f32, tag="acc", name=f"bps_{b}")
                nc.tensor.matmul(bps, lhsT=ones_sb[0:1, :], rhs=gm,
                                 start=True, stop=True)
                negmax = small_pool.tile([P, 1], f32, tag="negmax")
                nc.scalar.mul(out=negmax, in_=bps, mul=-1.0)

                for it in range(NT):
                    nc.scalar.activation(
                        out=agg[:, it, :],
                        in_=agg[:, it, :],
                        func=mybir.ActivationFunctionType.Exp,
                        bias=negmax,
                        scale=1.0,
                        accum_out=sums[:, it : it + 1],
                    )
                s1 = small_pool.tile([P, 1], f32, tag="s1")
                nc.vector.tensor_reduce(
                    out=s1, in_=sums, axis=mybir.AxisListType.X,
                    op=mybir.AluOpType.add,
                )
                sps = acc_pool.tile([P, 1], f32, tag="acc", name=f"sps_{b}")
                nc.tensor.matmul(sps, lhsT=ones_sb, rhs=s1, start=True, stop=True)
                rinv = small_pool.tile([P, 1], f32, tag="rinv")
                nc.vector.reciprocal(out=rinv, in_=sps)

                for it in range(NT):
                    nc.scalar.activation(
                        out=agg[:, it, :],
                        in_=agg[:, it, :],
                        func=mybir.ActivationFunctionType.Copy,
                        bias=0.0,
                        scale=rinv,
                    )
                dst = y[b].rearrange("(p t) j -> p t j", p=P)
                nc.gpsimd.dma_start(out=dst, in_=agg)

    nc.finalize()  # Bacc: register alloc, nop/event-sem legalization, ISA codegen
    return nc


def kernel(mha_masks) -> np.ndarray:
    global LAST_RESULT
    from concourse.bass_utils import run_bass_kernel_spmd

    xfull = np.ascontiguousarray(np.asarray(mha_masks, dtype=np.float32))
    assert xfull.shape == (B, H, N, N), xfull.shape

    nc = build_program()
    ident = np.eye(P, dtype=np.float32)
    in_maps = [
        {"x": xfull[i * BPC : (i + 1) * BPC], "ident": ident}
        for i in range(NCORES)
    ]
    import os

    kw = {}
    if os.environ.get("KERNEL_TRACE_DIR"):
        kw = dict(trace=True, tmpdir=os.environ["KERNEL_TRACE_DIR"])
    res = run_bass_kernel_spmd(nc, in_maps, core_ids=list(range(NCORES)), **kw)
    LAST_RESULT = res
    out = np.concatenate(
        [np.asarray(r["y"], dtype=np.float32) for r in res.results], axis=0
    )
    return out
